# revision 58
# baseline (speedup 1.0000x reference)
"""Trainium2 Bass kernel for an RNN-T style joint network MLP.

  out[b,t,u,o] = tanh(enc[b,t,:] @ W1[:512] + dec[b,u,:] @ W1[512:] + b1) @ W2 + b2

Shapes: enc (8, 256, 512), dec (8, 64, 512), W1 (1024, 1024), b1 (1024,),
W2 (1024, 128), b2 (128,), out (8, 256, 64, 128), all float32.

Sharding: data-parallel over batch - one batch element per NeuronCore,
no collectives.

Engine assignment (steady state is ACT-tanh roofline paced: 16.8M
tanh/core at 1 elem/cycle/lane @1.2GHz ~= 112us):
  - ACT: pure tanh in steady state; head-only PSUM evacuations of the
    first GEMMs (it idles there anyway until blk0's sums exist).
  - DVE: broadcast adds for 7 of 8 h-chunks (tensor_scalar runs at
    2x_1p: the [128,1] scalar pointer occupies read port 1, blocking
    the 2-port 4x mode) + all main-GEMM PSUM evacuations with +b2
    fused, writing bf16 (host upcasts).
  - PE:  enc/dec projections, main GEMM (bf16, N=512 into one PSUM
    bank), AND the 8th h-chunk's broadcast add for blocks 1..15:
    po[h,(u,t)] = I128 @ e7rep + biasT2_slab @ sel4 (K=4 selector
    matmul adds the per-u dec bias rows; slabs start at partition 0
    because matmul requires base_partition in {0,32,64}). ACT tanh's
    that chunk straight from PSUM (172cyc overhead vs 224 SBUF).
Input DMAs are issued from two sequencers in parallel (sync + the
otherwise-idle Pool) and chunked so the first GEMMs start early.

Measured (8 axon trn2 cores): ~147.5us HW exec, rel err ~4.2e-3
(bf16 datapath + bf16 output rounding; tolerance 2e-2). Breakdown:
~6.8us fixed runtime boot, ~14us head (2.6MB input DMA + first GEMMs
+ blk0 fill), ~108us ACT-paced steady (15 blocks x 7.16us), ~2us
drain, ~5us fixed epilogue. fp8 (which would halve PE time) measured
2.3-4.3e-2 rel err on this data - over tolerance, rejected.
"""

import os
import numpy as np
import ml_dtypes

B, T, U, D, H, O = 8, 256, 64, 512, 1024, 128
NCORES = 8
UB = 4            # u-block size (pipeline granularity)
HC = H // 128     # 8 h-chunks
NB = U // UB      # 16 u-blocks
NP = UB // 2      # u-pairs per block
OFF = int(os.environ.get("KERNEL_OFFLOAD", "1"))  # PE add-offload on/off

_CACHE = {}
LAST_RESULT = None  # BassKernelResults from the most recent run (for profiling)


def _build_program():
    from concourse import bacc, tile
    import concourse.mybir as mybir

    dt = mybir.dt
    f32, bf16 = dt.float32, dt.bfloat16
    Act = mybir.ActivationFunctionType

    nc = bacc.Bacc("TRN2", target_bir_lowering=False, debug=False)

    encT = nc.dram_tensor("encT", [D, T], bf16, kind="ExternalInput").ap()
    decT = nc.dram_tensor("decT", [D, U], bf16, kind="ExternalInput").ap()
    W1 = nc.dram_tensor("W1", [2 * D, H], bf16, kind="ExternalInput").ap()
    W2bf = nc.dram_tensor("W2bf", [H, O], bf16, kind="ExternalInput").ap()
    b1r = nc.dram_tensor("b1r", [128, HC], f32, kind="ExternalInput").ap()
    b2c = nc.dram_tensor("b2c", [O, 1], f32, kind="ExternalInput").ap()
    b1rep = nc.dram_tensor("b1rep", [1, 512], bf16, kind="ExternalInput").ap()
    i128 = nc.dram_tensor("i128", [128, 128], bf16, kind="ExternalInput").ap()
    sel4 = nc.dram_tensor("sel4", [UB, UB * T], bf16, kind="ExternalInput").ap()
    outT = nc.dram_tensor("outT", [O, U, T], bf16, kind="ExternalOutput").ap()

    NHD = HC - 1 if OFF else HC   # h-chunks handled by DVE adds per block
    BW = NHD * UB * T             # sum tile width (hc-major: [hc][u][t])
    FW = HC * UB * T              # tanh tile width (always all 8 hc)

    with tile.TileContext(nc) as tc:
        with tc.tile_pool(name="persist", bufs=1) as persist, \
             tc.tile_pool(name="sums", bufs=4) as sums_pool, \
             tc.tile_pool(name="tanhp", bufs=3) as tanh_pool, \
             tc.tile_pool(name="outsb", bufs=4) as out_pool, \
             tc.tile_pool(name="psum", bufs=4, space="PSUM") as psum_pool, \
             tc.tile_pool(name="psoff", bufs=2, space="PSUM") as off_pool:

            w1_sb = persist.tile([128, 8 * H], bf16, tag="w1")
            encT_sb = persist.tile([128, 4 * T], bf16, tag="encT")
            decT_sb = persist.tile([128, 4 * U], bf16, tag="decT")
            w2_sb = persist.tile([128, HC * O], bf16, tag="w2")
            b1_sb = persist.tile([128, HC], f32, tag="b1")
            b2_sb = persist.tile([128, 1], f32, tag="b2")
            e_sb = persist.tile([128, HC * T], bf16, tag="eproj")
            bias_sb = persist.tile([128, HC * U], f32, tag="bias")
            b1rep_sb = persist.tile([1, 512], bf16, tag="b1rep")
            i128_sb = persist.tile([128, 128], bf16, tag="i128")
            sel4_sb = persist.tile([UB, UB * T], bf16, tag="sel4")
            # biasT2[ul, blk*128+h'] = dec_proj[4*blk+ul, h-chunk-7] + b1
            # (partition = u-within-block so per-block lhsT slices start at
            # partition 0 -- matmul requires base_partition in {0,32,64})
            biasT2_sb = persist.tile([UB, NB * 128], bf16, tag="biasT2")
            # h-chunk-7 of e_proj replicated UB times (plain 2D rhs for the
            # per-block broadcast matmul; stride-0 rhs isn't valid MM ISA)
            e7rep_sb = persist.tile([128, UB * T], bf16, tag="e7rep")

            # ---- loads. Two issuing engines in parallel: sync carries the
            # head-critical stream in first-use order (hc01's weights gate
            # the first evacs/adds); the idle Pool (gpsimd) sequencer
            # carries the later W1 chunks.
            w1e = w1_sb[:, 0:4 * H].rearrange("p (c h) -> p c h", c=4)
            w1d = w1_sb[:, 4 * H:8 * H].rearrange("p (c h) -> p c h", c=4)

            def dma_w1(eng, enc_half, q):
                cs = q * 256                 # hc pair q
                dst = w1e if enc_half else w1d
                r0 = 0 if enc_half else 512
                eng.dma_start(
                    dst[:, :, cs:cs + 256],
                    W1[r0:r0 + 512, cs:cs + 256]
                    .rearrange("(c p) h -> p c h", p=128))

            nc.sync.dma_start(
                encT_sb[:, :].rearrange("p (c t) -> p c t", c=4),
                encT[:, :].rearrange("(c p) t -> p c t", p=128))
            nc.sync.dma_start(
                decT_sb[:, :].rearrange("p (c u) -> p c u", c=4),
                decT[:, :].rearrange("(c p) u -> p c u", p=128))
            nc.sync.dma_start(b1_sb[:], b1r[:, :])
            nc.sync.dma_start(
                w2_sb[:, :].rearrange("p (c o) -> p c o", c=HC),
                W2bf[:, :].rearrange("(c p) o -> p c o", p=128))
            nc.sync.dma_start(b1rep_sb[:], b1rep[:, :])
            nc.sync.dma_start(i128_sb[:], i128[:, :])
            nc.sync.dma_start(sel4_sb[:], sel4[:, :])
            nc.sync.dma_start(b2_sb[:], b2c[:, :])
            for q in range(4):
                dma_w1(nc.gpsimd, True, q)
                dma_w1(nc.gpsimd, False, q)

            # ---- head: first GEMMs, interleaved per h-chunk so PSUM slots
            # recycle; all evacuations on ACT (idle anyway until blk0's
            # sums exist).
            # enc: e_projT[h,t] = sum_d W_enc[d,h]*encT[d,t]
            # dec: bias[h,u] = sum_d W_dec[d,h]*decT[d,u] + b1[h]
            for hc in range(HC):
                pe = psum_pool.tile([128, T], f32, tag="ps", name=f"pe{hc}")
                for dc in range(4):
                    nc.tensor.matmul(
                        pe[:],
                        lhsT=w1_sb[:, dc * H + hc * 128: dc * H + hc * 128 + 128],
                        rhs=encT_sb[:, dc * T:(dc + 1) * T],
                        start=(dc == 0), stop=(dc == 3),
                    )
                nc.scalar.activation(e_sb[:, hc * T:(hc + 1) * T], pe[:],
                                     Act.Identity)

                pd = psum_pool.tile([128, U], f32, tag="ps", name=f"pd{hc}")
                for dc in range(4):
                    nc.tensor.matmul(
                        pd[:],
                        lhsT=w1_sb[:, (4 + dc) * H + hc * 128: (4 + dc) * H + hc * 128 + 128],
                        rhs=decT_sb[:, dc * U:(dc + 1) * U],
                        start=(dc == 0), stop=(dc == 3),
                    )
                nc.scalar.activation(bias_sb[:, hc * U:(hc + 1) * U], pd[:],
                                     Act.Identity, bias=b1_sb[:, hc:hc + 1])

            # blk0: all 8 h-chunks on DVE (the PE offload path's biasT2
            # isn't built yet), tanh in quarters for a fast pipeline fill
            sum0_sb = sums_pool.tile([128, HC * UB * T], bf16, tag="sum")
            tanh0_sb = tanh_pool.tile([128, FW], bf16, tag="tanh")
            for hc in range(HC):
                for ul in range(UB):
                    nc.vector.tensor_scalar_add(
                        sum0_sb[:, hc * (UB * T) + ul * T: hc * (UB * T) + ul * T + T],
                        e_sb[:, hc * T:(hc + 1) * T],
                        bias_sb[:, hc * U + ul: hc * U + ul + 1],
                    )
            if OFF:
                # e7rep feeds blk1+'s PE broadcast matmul
                for ul in range(UB):
                    nc.vector.tensor_copy(
                        e7rep_sb[:, ul * T:(ul + 1) * T],
                        e_sb[:, (HC - 1) * T:HC * T])
            for q in range(4):
                nc.scalar.activation(
                    tanh0_sb[:, q * 2048:(q + 1) * 2048],
                    sum0_sb[:, q * 2048:(q + 1) * 2048], Act.Tanh)

            def emit_biasT2():
                # biasT2[ul, blk*128+h'] = dec[u,:] @ W_dec[:, hc7] + b1 for
                # u = 4*blk+ul: per-block K=4 lhsT slabs starting at
                # partition 0 (matmul needs base_partition in {0,32,64}).
                h7 = (HC - 1) * 128
                for half in range(2):
                    pt = off_pool.tile([UB, 8 * 128], f32, tag="off",
                                       name=f"pt{half}")
                    for bl in range(8):
                        bk = half * 8 + bl
                        for dc in range(4):
                            nc.tensor.matmul(
                                pt[:, bl * 128:(bl + 1) * 128],
                                lhsT=decT_sb[:, dc * U + bk * UB: dc * U + bk * UB + UB],
                                rhs=w1_sb[:, (4 + dc) * H + h7:(4 + dc) * H + h7 + 128],
                                start=(dc == 0), stop=False,
                            )
                        nc.tensor.matmul(
                            pt[:, bl * 128:(bl + 1) * 128],
                            lhsT=sel4_sb[0:1, 0:UB],
                            rhs=b1rep_sb[0:1, (bl % 4) * 128:(bl % 4) * 128 + 128],
                            start=False, stop=True,
                        )
                    # evac on DVE (a f32->bf16 cast copy): keeps ACT's
                    # stream free to run straight into steady tanh
                    nc.vector.tensor_copy(
                        biasT2_sb[:, half * 8 * 128:(half + 1) * 8 * 128],
                        pt[:])

            if OFF:
                # emitted here so the builder's matmuls sit between the
                # first GEMMs and blk0's tanh-chasing GEMM in the in-order
                # PE stream (all its inputs land early)
                emit_biasT2()

            # blk0 main GEMM + evac + store
            pos = [psum_pool.tile([128, 2 * T], f32, tag="ps", name=f"po0_{p}")
                   for p in range(NP)]
            for hc in range(HC):
                for p in range(NP):
                    nc.tensor.matmul(
                        pos[p][:],
                        lhsT=w2_sb[:, hc * O:(hc + 1) * O],
                        rhs=tanh0_sb[:, hc * (UB * T) + p * 2 * T: hc * (UB * T) + (p + 1) * 2 * T],
                        start=(hc == 0), stop=(hc == HC - 1),
                    )
            out_sb = out_pool.tile([128, UB * T], bf16, tag="osb")
            for p in range(NP):
                nc.vector.tensor_scalar_add(
                    out_sb[:, p * 2 * T:(p + 1) * 2 * T], pos[p][:],
                    b2_sb[:, 0:1])
            nc.sync.dma_start(outT[:, 0:UB, :], out_sb[:])

            # ---- main pipeline over u-blocks 1..NB-1 ----
            # tanh layout per block: [hc][u][t] hc-major so a u-pair is
            # contiguous and the main GEMM runs N=512 into one PSUM bank.
            # With OFF: h-chunk 7 is computed by PE into PSUM (po) and
            # tanh'd from there; DVE adds cover h-chunks 0..6.
            for blk in range(1, NB):
                off_blk = OFF
                nhd = NHD if off_blk else HC

                po = None
                if off_blk:
                    po = off_pool.tile([128, UB * T], f32, tag="off",
                                       name=f"po{blk}")
                    # e-broadcast: po[h,(u,t)] = e[h,t] for each of UB u's
                    # (split in halves: one matmul may write at most one
                    # 512-fp32 PSUM bank)
                    for hf in range(2):
                        nc.tensor.matmul(
                            po[:, hf * 512:(hf + 1) * 512],
                            lhsT=i128_sb[:, :],
                            rhs=e7rep_sb[:, hf * 512:(hf + 1) * 512],
                            start=True, stop=False,
                        )
                        # + bias rows for u in this block (K=UB selector)
                        nc.tensor.matmul(
                            po[:, hf * 512:(hf + 1) * 512],
                            lhsT=biasT2_sb[:, blk * 128:(blk + 1) * 128],
                            rhs=sel4_sb[:, hf * 512:(hf + 1) * 512],
                            start=False, stop=True,
                        )

                sum_sb = sums_pool.tile([128, HC * UB * T], bf16, tag="sum")
                for hc in range(nhd):
                    for ul in range(UB):
                        u = blk * UB + ul
                        nc.vector.tensor_scalar_add(
                            sum_sb[:, hc * (UB * T) + ul * T: hc * (UB * T) + ul * T + T],
                            e_sb[:, hc * T:(hc + 1) * T],
                            bias_sb[:, hc * U + u: hc * U + u + 1],
                        )

                tanh_sb = tanh_pool.tile([128, FW], bf16, tag="tanh")
                if off_blk:
                    # offloaded chunk first: PE produces it early, and the
                    # main GEMM consumes hc7 first (hc order [7,0..6])
                    nc.scalar.activation(
                        tanh_sb[:, (HC - 1) * (UB * T):], po[:], Act.Tanh)
                sw = nhd * UB * T
                if blk == NB - 1:
                    # piece-split tanh at the drain so the PE can chase; the
                    # final piece is a 512-wide half-chunk (= the last
                    # GEMM pair's rhs) to shorten the post-tanh tail
                    qs = [0, 2048, 4096, 6144, 6656, sw]
                    for q in range(len(qs) - 1):
                        nc.scalar.activation(
                            tanh_sb[:, qs[q]:qs[q + 1]],
                            sum_sb[:, qs[q]:qs[q + 1]], Act.Tanh)
                else:
                    nc.scalar.activation(tanh_sb[:, 0:sw], sum_sb[:, 0:sw],
                                         Act.Tanh)

                pos = [psum_pool.tile([128, 2 * T], f32, tag="ps",
                                      name=f"po{blk}_{p}")
                       for p in range(NP)]
                hc_order = ([HC - 1] + list(range(HC - 1))) if off_blk \
                    else list(range(HC))
                for i, hc in enumerate(hc_order):  # W2 chunk stays stationary
                    for p in range(NP):
                        nc.tensor.matmul(
                            pos[p][:],
                            lhsT=w2_sb[:, hc * O:(hc + 1) * O],
                            rhs=tanh_sb[:, hc * (UB * T) + p * 2 * T: hc * (UB * T) + (p + 1) * 2 * T],
                            start=(i == 0), stop=(i == HC - 1),
                        )

                out_sb = out_pool.tile([128, UB * T], bf16, tag="osb")
                if blk == NB - 1:
                    # final evacs on both engines in parallel (ACT is idle
                    # after its last tanh); store each pair as soon as its
                    # evac lands
                    nc.vector.tensor_scalar_add(
                        out_sb[:, 0:2 * T], pos[0][:], b2_sb[:, 0:1])
                    nc.sync.dma_start(outT[:, blk * UB:blk * UB + 2, :],
                                      out_sb[:, 0:2 * T])
                    nc.scalar.activation(
                        out_sb[:, 2 * T:4 * T], pos[1][:],
                        Act.Identity, bias=b2_sb[:, 0:1])
                    nc.sync.dma_start(outT[:, blk * UB + 2:(blk + 1) * UB, :],
                                      out_sb[:, 2 * T:])
                else:
                    for p in range(NP):
                        nc.vector.tensor_scalar_add(
                            out_sb[:, p * 2 * T:(p + 1) * 2 * T], pos[p][:],
                            b2_sb[:, 0:1])
                    nc.sync.dma_start(outT[:, blk * UB:(blk + 1) * UB, :],
                                      out_sb[:])

    nc.compile()
    return nc


def kernel(encoder_state, decoder_state, W1, b1, W2, b2):
    from concourse.bass_utils import run_bass_kernel_spmd
    global LAST_RESULT

    if "nc" not in _CACHE:
        _CACHE["nc"] = _build_program()
    nc = _CACHE["nc"]

    encoder_state = np.asarray(encoder_state, dtype=np.float32)
    decoder_state = np.asarray(decoder_state, dtype=np.float32)
    W1 = np.asarray(W1, dtype=np.float32)
    b1 = np.asarray(b1, dtype=np.float32)
    W2 = np.asarray(W2, dtype=np.float32)
    b2 = np.asarray(b2, dtype=np.float32)

    bf = ml_dtypes.bfloat16
    W1bf = W1.astype(bf)
    W2bf = W2.astype(bf)
    b1r = np.ascontiguousarray(b1.reshape(HC, 128).T)  # [128, 8]
    b2c = np.ascontiguousarray(b2.reshape(O, 1))
    b1rep = np.tile(b1[(HC - 1) * 128:], 4).reshape(1, 512).astype(bf)
    i128 = np.eye(128, dtype=np.float32).astype(bf)
    sel4 = np.kron(np.eye(UB, dtype=np.float32),
                   np.ones((1, T), dtype=np.float32)).astype(bf)

    in_maps = []
    for i in range(NCORES):
        in_maps.append({
            "encT": np.ascontiguousarray(encoder_state[i].T.astype(bf)),  # [512, 256]
            "decT": np.ascontiguousarray(decoder_state[i].T.astype(bf)),  # [512, 64]
            "W1": W1bf,
            "W2bf": W2bf,
            "b1r": b1r,
            "b2c": b2c,
            "b1rep": b1rep,
            "i128": i128,
            "sel4": sel4,
        })

    trace = bool(int(os.environ.get("KERNEL_TRACE", "0")))
    res = run_bass_kernel_spmd(nc, in_maps, list(range(NCORES)), trace=trace)
    LAST_RESULT = res

    # gather: outT[core] is [O, U, T] bf16 -> out[b, t, u, o] f32
    out = np.empty((B, T, U, O), dtype=np.float32)
    for i in range(NCORES):
        out[i] = res.results[i]["outT"].astype(np.float32).transpose(2, 1, 0)
    return out


# revision 60
# speedup vs baseline: 1.0060x; 1.0060x over previous
"""Trainium2 Bass kernel for an RNN-T style joint network MLP.

  out[b,t,u,o] = tanh(enc[b,t,:] @ W1[:512] + dec[b,u,:] @ W1[512:] + b1) @ W2 + b2

Shapes: enc (8, 256, 512), dec (8, 64, 512), W1 (1024, 1024), b1 (1024,),
W2 (1024, 128), b2 (128,), out (8, 256, 64, 128), all float32.

Sharding: data-parallel over batch - one batch element per NeuronCore,
no collectives.

Engine assignment (steady state is ACT-tanh roofline paced: 16.8M
tanh/core at 1 elem/cycle/lane @1.2GHz ~= 112us):
  - ACT: pure tanh in steady state; head-only PSUM evacuations of the
    first GEMMs (it idles there anyway until blk0's sums exist).
  - DVE: broadcast adds for 7 of 8 h-chunks (tensor_scalar runs at
    2x_1p: the [128,1] scalar pointer occupies read port 1, blocking
    the 2-port 4x mode) + all main-GEMM PSUM evacuations with +b2
    fused, writing bf16 (host upcasts).
  - PE:  enc/dec projections, main GEMM (bf16, N=512 into one PSUM
    bank), AND the 8th h-chunk's broadcast add for blocks 1..15:
    po[h,(u,t)] = I128 @ e7rep + biasT2_slab @ sel4 (K=4 selector
    matmul adds the per-u dec bias rows; slabs start at partition 0
    because matmul requires base_partition in {0,32,64}). ACT tanh's
    that chunk straight from PSUM (172cyc overhead vs 224 SBUF).
Input DMAs are issued from two sequencers in parallel (sync + the
otherwise-idle Pool) and chunked so the first GEMMs start early.

Measured (8 axon trn2 cores): ~147.5us HW exec, rel err ~4.2e-3
(bf16 datapath + bf16 output rounding; tolerance 2e-2). Breakdown:
~6.8us fixed runtime boot, ~14us head (2.6MB input DMA + first GEMMs
+ blk0 fill), ~108us ACT-paced steady (15 blocks x 7.16us), ~2us
drain, ~5us fixed epilogue. fp8 (which would halve PE time) measured
2.3-4.3e-2 rel err on this data - over tolerance, rejected.
"""

import os
import numpy as np
import ml_dtypes

B, T, U, D, H, O = 8, 256, 64, 512, 1024, 128
NCORES = 8
UB = 4            # u-block size (pipeline granularity)
HC = H // 128     # 8 h-chunks
NB = U // UB      # 16 u-blocks
NP = UB // 2      # u-pairs per block
OFF = int(os.environ.get("KERNEL_OFFLOAD", "1"))  # PE add-offload on/off

_CACHE = {}
LAST_RESULT = None  # BassKernelResults from the most recent run (for profiling)


def _build_program():
    from concourse import bacc, tile
    import concourse.mybir as mybir

    dt = mybir.dt
    f32, bf16 = dt.float32, dt.bfloat16
    Act = mybir.ActivationFunctionType

    nc = bacc.Bacc("TRN2", target_bir_lowering=False, debug=False)

    encT = nc.dram_tensor("encT", [D, T], bf16, kind="ExternalInput").ap()
    decT = nc.dram_tensor("decT", [D, U], bf16, kind="ExternalInput").ap()
    W1 = nc.dram_tensor("W1", [2 * D, H], bf16, kind="ExternalInput").ap()
    W2bf = nc.dram_tensor("W2bf", [H, O], bf16, kind="ExternalInput").ap()
    b1r = nc.dram_tensor("b1r", [128, HC], f32, kind="ExternalInput").ap()
    b2c = nc.dram_tensor("b2c", [O, 1], f32, kind="ExternalInput").ap()
    b1rep = nc.dram_tensor("b1rep", [1, 512], bf16, kind="ExternalInput").ap()
    i128 = nc.dram_tensor("i128", [128, 128], bf16, kind="ExternalInput").ap()
    sel4 = nc.dram_tensor("sel4", [UB, UB * T], bf16, kind="ExternalInput").ap()
    outT = nc.dram_tensor("outT", [O, U, T], bf16, kind="ExternalOutput").ap()

    NHD = HC - 1 if OFF else HC   # h-chunks handled by DVE adds per block
    BW = NHD * UB * T             # sum tile width (hc-major: [hc][u][t])
    FW = HC * UB * T              # tanh tile width (always all 8 hc)

    with tile.TileContext(nc) as tc:
        with tc.tile_pool(name="persist", bufs=1) as persist, \
             tc.tile_pool(name="sums", bufs=4) as sums_pool, \
             tc.tile_pool(name="tanhp", bufs=3) as tanh_pool, \
             tc.tile_pool(name="outsb", bufs=4) as out_pool, \
             tc.tile_pool(name="psum", bufs=4, space="PSUM") as psum_pool, \
             tc.tile_pool(name="psoff", bufs=2, space="PSUM") as off_pool:

            w1_sb = persist.tile([128, 8 * H], bf16, tag="w1")
            encT_sb = persist.tile([128, 4 * T], bf16, tag="encT")
            decT_sb = persist.tile([128, 4 * U], bf16, tag="decT")
            w2_sb = persist.tile([128, HC * O], bf16, tag="w2")
            b1_sb = persist.tile([128, HC], f32, tag="b1")
            b2_sb = persist.tile([128, 1], f32, tag="b2")
            e_sb = persist.tile([128, HC * T], bf16, tag="eproj")
            bias_sb = persist.tile([128, HC * U], f32, tag="bias")
            b1rep_sb = persist.tile([1, 512], bf16, tag="b1rep")
            i128_sb = persist.tile([128, 128], bf16, tag="i128")
            sel4_sb = persist.tile([UB, UB * T], bf16, tag="sel4")
            # biasT2[ul, blk*128+h'] = dec_proj[4*blk+ul, h-chunk-7] + b1
            # (partition = u-within-block so per-block lhsT slices start at
            # partition 0 -- matmul requires base_partition in {0,32,64})
            biasT2_sb = persist.tile([UB, NB * 128], bf16, tag="biasT2")
            # h-chunk-7 of e_proj replicated UB times (plain 2D rhs for the
            # per-block broadcast matmul; stride-0 rhs isn't valid MM ISA)
            e7rep_sb = persist.tile([128, UB * T], bf16, tag="e7rep")

            # ---- loads. Two issuing engines in parallel: sync carries the
            # head-critical stream in first-use order (hc01's weights gate
            # the first evacs/adds); the idle Pool (gpsimd) sequencer
            # carries the later W1 chunks.
            # one DMA per hc-pair carries BOTH the enc and dec weight
            # chunks: W1's rows are enc then dec and w1_sb's layout is
            # [enc c0..3 | dec c4..7], so the combined [p, 8, 256] pattern
            # is contiguous -- 4 issues instead of 8 halves the gpsimd
            # issue tail that gates hc7's GEMMs and blk0's last quarter
            w1all = w1_sb[:, :].rearrange("p (c h) -> p c h", c=8)

            nc.sync.dma_start(
                encT_sb[:, :].rearrange("p (c t) -> p c t", c=4),
                encT[:, :].rearrange("(c p) t -> p c t", p=128))
            nc.sync.dma_start(
                decT_sb[:, :].rearrange("p (c u) -> p c u", c=4),
                decT[:, :].rearrange("(c p) u -> p c u", p=128))
            nc.sync.dma_start(b1_sb[:], b1r[:, :])
            nc.sync.dma_start(
                w2_sb[:, :].rearrange("p (c o) -> p c o", c=HC),
                W2bf[:, :].rearrange("(c p) o -> p c o", p=128))
            nc.sync.dma_start(b1rep_sb[:], b1rep[:, :])
            nc.sync.dma_start(i128_sb[:], i128[:, :])
            nc.sync.dma_start(sel4_sb[:], sel4[:, :])
            nc.sync.dma_start(b2_sb[:], b2c[:, :])
            for q in range(4):
                cs = q * 256
                nc.gpsimd.dma_start(
                    w1all[:, :, cs:cs + 256],
                    W1[:, cs:cs + 256].rearrange("(c p) h -> p c h", p=128))

            # ---- head: first GEMMs, interleaved per h-chunk so PSUM slots
            # recycle; all evacuations on ACT (idle anyway until blk0's
            # sums exist).
            # enc: e_projT[h,t] = sum_d W_enc[d,h]*encT[d,t]
            # dec: bias[h,u] = sum_d W_dec[d,h]*decT[d,u] + b1[h]
            for hc in range(HC):
                pe = psum_pool.tile([128, T], f32, tag="ps", name=f"pe{hc}")
                for dc in range(4):
                    nc.tensor.matmul(
                        pe[:],
                        lhsT=w1_sb[:, dc * H + hc * 128: dc * H + hc * 128 + 128],
                        rhs=encT_sb[:, dc * T:(dc + 1) * T],
                        start=(dc == 0), stop=(dc == 3),
                    )
                nc.scalar.activation(e_sb[:, hc * T:(hc + 1) * T], pe[:],
                                     Act.Identity)

                pd = psum_pool.tile([128, U], f32, tag="ps", name=f"pd{hc}")
                for dc in range(4):
                    nc.tensor.matmul(
                        pd[:],
                        lhsT=w1_sb[:, (4 + dc) * H + hc * 128: (4 + dc) * H + hc * 128 + 128],
                        rhs=decT_sb[:, dc * U:(dc + 1) * U],
                        start=(dc == 0), stop=(dc == 3),
                    )
                nc.scalar.activation(bias_sb[:, hc * U:(hc + 1) * U], pd[:],
                                     Act.Identity, bias=b1_sb[:, hc:hc + 1])

            # blk0: all 8 h-chunks on DVE (the PE offload path's biasT2
            # isn't built yet), tanh in quarters for a fast pipeline fill
            sum0_sb = sums_pool.tile([128, HC * UB * T], bf16, tag="sum")
            tanh0_sb = tanh_pool.tile([128, FW], bf16, tag="tanh")
            for hc in range(HC):
                for ul in range(UB):
                    nc.vector.tensor_scalar_add(
                        sum0_sb[:, hc * (UB * T) + ul * T: hc * (UB * T) + ul * T + T],
                        e_sb[:, hc * T:(hc + 1) * T],
                        bias_sb[:, hc * U + ul: hc * U + ul + 1],
                    )
            if OFF:
                # e7rep feeds blk1+'s PE broadcast matmul
                for ul in range(UB):
                    nc.vector.tensor_copy(
                        e7rep_sb[:, ul * T:(ul + 1) * T],
                        e_sb[:, (HC - 1) * T:HC * T])
            for q in range(4):
                nc.scalar.activation(
                    tanh0_sb[:, q * 2048:(q + 1) * 2048],
                    sum0_sb[:, q * 2048:(q + 1) * 2048], Act.Tanh)

            def emit_biasT2():
                # biasT2[ul, blk*128+h'] = dec[u,:] @ W_dec[:, hc7] + b1 for
                # u = 4*blk+ul: per-block K=4 lhsT slabs starting at
                # partition 0 (matmul needs base_partition in {0,32,64}).
                h7 = (HC - 1) * 128
                for half in range(2):
                    pt = off_pool.tile([UB, 8 * 128], f32, tag="off",
                                       name=f"pt{half}")
                    for bl in range(8):
                        bk = half * 8 + bl
                        for dc in range(4):
                            nc.tensor.matmul(
                                pt[:, bl * 128:(bl + 1) * 128],
                                lhsT=decT_sb[:, dc * U + bk * UB: dc * U + bk * UB + UB],
                                rhs=w1_sb[:, (4 + dc) * H + h7:(4 + dc) * H + h7 + 128],
                                start=(dc == 0), stop=False,
                            )
                        nc.tensor.matmul(
                            pt[:, bl * 128:(bl + 1) * 128],
                            lhsT=sel4_sb[0:1, 0:UB],
                            rhs=b1rep_sb[0:1, (bl % 4) * 128:(bl % 4) * 128 + 128],
                            start=False, stop=True,
                        )
                    # evac on DVE (a f32->bf16 cast copy): keeps ACT's
                    # stream free to run straight into steady tanh
                    nc.vector.tensor_copy(
                        biasT2_sb[:, half * 8 * 128:(half + 1) * 8 * 128],
                        pt[:])

            if OFF:
                # emitted here so the builder's matmuls sit between the
                # first GEMMs and blk0's tanh-chasing GEMM in the in-order
                # PE stream (all its inputs land early)
                emit_biasT2()

            # blk0 main GEMM + evac + store
            pos = [psum_pool.tile([128, 2 * T], f32, tag="ps", name=f"po0_{p}")
                   for p in range(NP)]
            for hc in range(HC):
                for p in range(NP):
                    nc.tensor.matmul(
                        pos[p][:],
                        lhsT=w2_sb[:, hc * O:(hc + 1) * O],
                        rhs=tanh0_sb[:, hc * (UB * T) + p * 2 * T: hc * (UB * T) + (p + 1) * 2 * T],
                        start=(hc == 0), stop=(hc == HC - 1),
                    )
            out_sb = out_pool.tile([128, UB * T], bf16, tag="osb")
            for p in range(NP):
                nc.vector.tensor_scalar_add(
                    out_sb[:, p * 2 * T:(p + 1) * 2 * T], pos[p][:],
                    b2_sb[:, 0:1])
            nc.sync.dma_start(outT[:, 0:UB, :], out_sb[:])

            # ---- main pipeline over u-blocks 1..NB-1 ----
            # tanh layout per block: [hc][u][t] hc-major so a u-pair is
            # contiguous and the main GEMM runs N=512 into one PSUM bank.
            # With OFF: h-chunk 7 is computed by PE into PSUM (po) and
            # tanh'd from there; DVE adds cover h-chunks 0..6.
            for blk in range(1, NB):
                off_blk = OFF
                nhd = NHD if off_blk else HC

                po = None
                if off_blk:
                    po = off_pool.tile([128, UB * T], f32, tag="off",
                                       name=f"po{blk}")
                    # e-broadcast: po[h,(u,t)] = e[h,t] for each of UB u's
                    # (split in halves: one matmul may write at most one
                    # 512-fp32 PSUM bank)
                    for hf in range(2):
                        nc.tensor.matmul(
                            po[:, hf * 512:(hf + 1) * 512],
                            lhsT=i128_sb[:, :],
                            rhs=e7rep_sb[:, hf * 512:(hf + 1) * 512],
                            start=True, stop=False,
                        )
                        # + bias rows for u in this block (K=UB selector)
                        nc.tensor.matmul(
                            po[:, hf * 512:(hf + 1) * 512],
                            lhsT=biasT2_sb[:, blk * 128:(blk + 1) * 128],
                            rhs=sel4_sb[:, hf * 512:(hf + 1) * 512],
                            start=False, stop=True,
                        )

                sum_sb = sums_pool.tile([128, HC * UB * T], bf16, tag="sum")
                for hc in range(nhd):
                    for ul in range(UB):
                        u = blk * UB + ul
                        nc.vector.tensor_scalar_add(
                            sum_sb[:, hc * (UB * T) + ul * T: hc * (UB * T) + ul * T + T],
                            e_sb[:, hc * T:(hc + 1) * T],
                            bias_sb[:, hc * U + u: hc * U + u + 1],
                        )

                tanh_sb = tanh_pool.tile([128, FW], bf16, tag="tanh")
                if off_blk:
                    # offloaded chunk first: PE produces it early, and the
                    # main GEMM consumes hc7 first (hc order [7,0..6])
                    nc.scalar.activation(
                        tanh_sb[:, (HC - 1) * (UB * T):], po[:], Act.Tanh)
                sw = nhd * UB * T
                if blk == NB - 1:
                    # piece-split tanh at the drain so the PE can chase; the
                    # final piece is a 512-wide half-chunk (= the last
                    # GEMM pair's rhs) to shorten the post-tanh tail
                    qs = [0, 2048, 4096, 6144, 6656, sw]
                    for q in range(len(qs) - 1):
                        nc.scalar.activation(
                            tanh_sb[:, qs[q]:qs[q + 1]],
                            sum_sb[:, qs[q]:qs[q + 1]], Act.Tanh)
                else:
                    nc.scalar.activation(tanh_sb[:, 0:sw], sum_sb[:, 0:sw],
                                         Act.Tanh)

                pos = [psum_pool.tile([128, 2 * T], f32, tag="ps",
                                      name=f"po{blk}_{p}")
                       for p in range(NP)]
                hc_order = ([HC - 1] + list(range(HC - 1))) if off_blk \
                    else list(range(HC))
                for i, hc in enumerate(hc_order):  # W2 chunk stays stationary
                    for p in range(NP):
                        nc.tensor.matmul(
                            pos[p][:],
                            lhsT=w2_sb[:, hc * O:(hc + 1) * O],
                            rhs=tanh_sb[:, hc * (UB * T) + p * 2 * T: hc * (UB * T) + (p + 1) * 2 * T],
                            start=(i == 0), stop=(i == HC - 1),
                        )

                out_sb = out_pool.tile([128, UB * T], bf16, tag="osb")
                if blk == NB - 1:
                    # final evacs on both engines in parallel (ACT is idle
                    # after its last tanh); store each pair as soon as its
                    # evac lands
                    nc.vector.tensor_scalar_add(
                        out_sb[:, 0:2 * T], pos[0][:], b2_sb[:, 0:1])
                    nc.sync.dma_start(outT[:, blk * UB:blk * UB + 2, :],
                                      out_sb[:, 0:2 * T])
                    nc.scalar.activation(
                        out_sb[:, 2 * T:4 * T], pos[1][:],
                        Act.Identity, bias=b2_sb[:, 0:1])
                    nc.sync.dma_start(outT[:, blk * UB + 2:(blk + 1) * UB, :],
                                      out_sb[:, 2 * T:])
                else:
                    for p in range(NP):
                        nc.vector.tensor_scalar_add(
                            out_sb[:, p * 2 * T:(p + 1) * 2 * T], pos[p][:],
                            b2_sb[:, 0:1])
                    nc.sync.dma_start(outT[:, blk * UB:(blk + 1) * UB, :],
                                      out_sb[:])

    nc.compile()
    return nc


def kernel(encoder_state, decoder_state, W1, b1, W2, b2):
    from concourse.bass_utils import run_bass_kernel_spmd
    global LAST_RESULT

    if "nc" not in _CACHE:
        _CACHE["nc"] = _build_program()
    nc = _CACHE["nc"]

    encoder_state = np.asarray(encoder_state, dtype=np.float32)
    decoder_state = np.asarray(decoder_state, dtype=np.float32)
    W1 = np.asarray(W1, dtype=np.float32)
    b1 = np.asarray(b1, dtype=np.float32)
    W2 = np.asarray(W2, dtype=np.float32)
    b2 = np.asarray(b2, dtype=np.float32)

    bf = ml_dtypes.bfloat16
    W1bf = W1.astype(bf)
    W2bf = W2.astype(bf)
    b1r = np.ascontiguousarray(b1.reshape(HC, 128).T)  # [128, 8]
    b2c = np.ascontiguousarray(b2.reshape(O, 1))
    b1rep = np.tile(b1[(HC - 1) * 128:], 4).reshape(1, 512).astype(bf)
    i128 = np.eye(128, dtype=np.float32).astype(bf)
    sel4 = np.kron(np.eye(UB, dtype=np.float32),
                   np.ones((1, T), dtype=np.float32)).astype(bf)

    in_maps = []
    for i in range(NCORES):
        in_maps.append({
            "encT": np.ascontiguousarray(encoder_state[i].T.astype(bf)),  # [512, 256]
            "decT": np.ascontiguousarray(decoder_state[i].T.astype(bf)),  # [512, 64]
            "W1": W1bf,
            "W2bf": W2bf,
            "b1r": b1r,
            "b2c": b2c,
            "b1rep": b1rep,
            "i128": i128,
            "sel4": sel4,
        })

    trace = bool(int(os.environ.get("KERNEL_TRACE", "0")))
    res = run_bass_kernel_spmd(nc, in_maps, list(range(NCORES)), trace=trace)
    LAST_RESULT = res

    # gather: outT[core] is [O, U, T] bf16 -> out[b, t, u, o] f32
    out = np.empty((B, T, U, O), dtype=np.float32)
    for i in range(NCORES):
        out[i] = res.results[i]["outT"].astype(np.float32).transpose(2, 1, 0)
    return out
